# revision 27
# baseline (speedup 1.0000x reference)
"""CBAM (channel + spatial attention) Trainium2 kernel, 8-core data parallel.

Problem: f [8, 8, 256, 56, 56] f32 -> out same shape.
  x = f.reshape(BT, C, H, W)
  ca = sigmoid(mlp(max_hw(x)) + mlp(mean_hw(x)));  xc = ca * x
  s  = conv7x7([mean_c(xc); max_c(xc)]);           out = sigmoid(s) * xc

Strategy (per NeuronCore, 8 frames each, no collectives):
  - bf16 end-to-end on device: host casts f to bf16, device returns bf16,
    host upcasts -> halves HBM traffic (rel-err budget 2e-2 >> bf16 noise)
  - channel stats via DVE tensor_tensor_reduce: one pass per t computes
    (x_lo max x_hi) with fused max-accum -> pr_max; one TTR(add,add,
    scale=1/HW) -> pr_mean t0; t1 mean via ACT accum pass (engine balance)
  - m1 = max(ca1*x1, xc0) via fused scalar_tensor_tensor (xc1 never
    materialized); xc0 via ACT scaled copy
  - ssum: PE matmuls (ca_b^T x) into PSUM chunks at partitions {0,32,64}
    x 3 banks; GPS SWDGE scatters PSUM->conv rows with f32->bf16 cast
  - smax: m1 laid out [128, 3584] (28*128, pixel hw = 28p + j), DMA XBAR
    transpose (2 halves on SP/ACT queues) -> mT [128, 28, 128] SBUF bf16;
    DVE fold tree at 2x + small reduce -> smT [128, 28]; one affine
    scatter via (y h) j view = [64, 56]
  - conv 7x7 as 7 accumulating PE matmuls (banded lhsT, 1/C folded in)
  - sa broadcast via GPSIMD partition_broadcast in two halves
  - final: ob = (x*ca)*sab via 4 quarter scalar_tensor_tensor ops
  - stores: t0 on GPSIMD SWDGE, t1 on ACT HWDGE queue
"""

import sys
from contextlib import ExitStack

import numpy as np

if "/opt/trn_rl_repo" not in sys.path:
    sys.path.insert(0, "/opt/trn_rl_repo")

import concourse.bass as bass
import concourse.tile as tile
from concourse import bacc, mybir
from concourse.bass_utils import run_bass_kernel_spmd
from concourse.masks import make_identity

F32 = mybir.dt.float32
BF16 = mybir.dt.bfloat16
ALU = mybir.AluOpType
ACTF = mybir.ActivationFunctionType

N_CORES = 8
B, T, C, H, W = 8, 8, 256, 56, 56
HW = H * W            # 3136
FRAMES = B * T        # 64
FPC = FRAMES // N_CORES  # frames per core = 8
PAD = 3
HP, WP = H + 2 * PAD, W + 2 * PAD  # 62, 62
SCHK = 392            # ssum chunk width (8 chunks)
HHW = HW // 2         # 1568
MPAD = 28 * 128       # 3584: m1 padded so hw = 28*p + j transposes cleanly
NEG = -3.0e38


def _build_conv_lhsT(conv_w: np.ndarray) -> np.ndarray:
    """Banded matrices for the 7x7 conv as accumulating matmuls.

    Branch 0 (avg, y-banded, 1/C folded): B[0, dx][yi, yo] =
    w_eff[0, yi-yo, dx] -- contracts y_in for each of 7 dx columns.
    Branch 1 (max, x-banded, operates on the transposed max map):
    B[1, dy][xi, xo] = w_eff[1, dy, xi-xo] -- contracts x_in for each
    of 7 dy rows; its [x_out, y_out] result is transposed back on PE.
    """
    w_eff = conv_w[0].astype(np.float64).copy()  # [2, 7, 7]
    w_eff[0] /= C
    Bm = np.zeros((2, 7, HP, H), dtype=np.float32)
    di = np.arange(7)
    for d in range(7):
        for o in range(H):
            Bm[0, d, o + di, o] = w_eff[0, :, d]   # dx = d, band over y
            Bm[1, d, o + di, o] = w_eff[1, d, :]   # dy = d, band over x
    return Bm


def build_nc(n_frames: int = FPC):
    nc = bacc.Bacc("TRN2", target_bir_lowering=False, debug=False,
                   num_devices=N_CORES)

    x_ext = nc.dram_tensor("x", [n_frames, C, HW], BF16, kind="ExternalInput")
    w1_ext = nc.dram_tensor("w1", [C, 16], F32, kind="ExternalInput")
    w2_ext = nc.dram_tensor("w2", [16, C], F32, kind="ExternalInput")
    cb_ext = nc.dram_tensor("convb", [2, 7, HP, H], F32, kind="ExternalInput")
    out_ext = nc.dram_tensor("out", [n_frames, C, HW], BF16,
                             kind="ExternalOutput")

    with tile.TileContext(nc) as tc, ExitStack() as ctx:
        consts = ctx.enter_context(tc.tile_pool(name="consts", bufs=1))
        xin = ctx.enter_context(tc.tile_pool(name="xin", bufs=3))
        scrp = ctx.enter_context(tc.tile_pool(name="scr", bufs=1))
        xcp = ctx.enter_context(tc.tile_pool(name="xc", bufs=2))
        m1p = ctx.enter_context(tc.tile_pool(name="m1", bufs=2))
        mtp = ctx.enter_context(tc.tile_pool(name="mt", bufs=2))
        foldp = ctx.enter_context(tc.tile_pool(name="fold", bufs=1))
        smtp = ctx.enter_context(tc.tile_pool(name="smt", bufs=2))
        sabp = ctx.enter_context(tc.tile_pool(name="sab", bufs=2))
        obp = ctx.enter_context(tc.tile_pool(name="ob", bufs=2))
        sap = ctx.enter_context(tc.tile_pool(name="sa", bufs=2))
        small = ctx.enter_context(tc.tile_pool(name="small", bufs=3))
        # PSUM: pss 4 banks + misc 2 banks = 6
        pssp = ctx.enter_context(tc.tile_pool(name="pss", bufs=1, space="PSUM"))
        pmp = ctx.enter_context(tc.tile_pool(name="pm", bufs=2, space="PSUM"))

        # ---- constants / weights (loaded once) ----
        w1_sb = consts.tile([128, 2, 16], F32)       # [k, ktile, m]
        for t in range(2):
            nc.sync.dma_start(w1_sb[:, t, :], w1_ext[t * 128:(t + 1) * 128, :])
        w2_sb = consts.tile([16, C], F32)
        nc.sync.dma_start(w2_sb[:], w2_ext[:, :])
        cb_sb = consts.tile([HP, 2, 7, H], BF16)     # [in_row, branch, d, out]
        nc.gpsimd.dma_start(                          # SWDGE: casts f32->bf16
            cb_sb[:],
            cb_ext.rearrange("b d p y -> p b d y"),
        )
        ident = consts.tile([64, 64], BF16)
        make_identity(nc, ident[:])

        xbs = {}

        def load_frame(g):
            if g >= n_frames:
                return
            xb = xin.tile([128, 2, HW], BF16, tag="x")
            for t in range(2):
                nc.sync.dma_start(
                    xb[:, t, :], x_ext[g, t * 128:(t + 1) * 128, :])
            xbs[g] = xb

        load_frame(0)
        load_frame(1)

        for f in range(n_frames):
            load_frame(f + 2)
            xb = xbs.pop(f)

            # ---------- stats: max fold tree (DVE); means via accumulate
            # (t0 on DVE tensor_scalar f32-accum, t1 on ACT -- balance) ---
            pr_max = small.tile([128, 2], F32, tag="prmax")
            pr_sum = small.tile([128, 2], F32, tag="prsum")
            f1 = foldp.tile([128, 2, HHW], BF16, tag="f1")
            nc.vector.tensor_tensor(
                out=f1[:], in0=xb[:, :, 0:HHW], in1=xb[:, :, HHW:HW],
                op=ALU.max)
            f2 = foldp.tile([128, 2, 784], BF16, tag="f2")
            nc.vector.tensor_tensor(
                out=f2[:], in0=f1[:, :, 0:784], in1=f1[:, :, 784:HHW],
                op=ALU.max)
            f3 = foldp.tile([128, 2, 392], BF16, tag="f3")
            nc.vector.tensor_tensor(
                out=f3[:], in0=f2[:, :, 0:392], in1=f2[:, :, 392:784],
                op=ALU.max)
            nc.vector.tensor_reduce(
                out=pr_max[:], in_=f3[:],
                axis=mybir.AxisListType.X, op=ALU.max)
            ts_scr = scrp.tile([128, HW], BF16, tag="ts_scr")
            nc.vector.tensor_scalar(
                out=ts_scr[:], in0=xb[:, 0, :], scalar1=1.0 / HW,
                scalar2=0.0, op0=ALU.mult, op1=ALU.add,
                accum_out=pr_sum[:, 0:1])
            scr1 = scrp.tile([128, HW], BF16, tag="scr1")
            nc.scalar.activation(
                scr1[:], xb[:, 1, :], ACTF.Copy, scale=1.0 / HW,
                accum_out=pr_sum[:, 1:2])

            # ---------- MLP on PE (stats are already mean-scaled) --------
            ph = pmp.tile([16, 2], F32, tag="misc")
            for si, prs in ((0, pr_max), (1, pr_sum)):
                for t in range(2):
                    nc.tensor.matmul(ph[:, si:si + 1], w1_sb[:, t, :],
                                     prs[:, t:t + 1],
                                     start=(t == 0), stop=(t == 1))
            h = small.tile([16, 2], F32, tag="h")
            nc.scalar.activation(h[:], ph[:], ACTF.Relu)
            hs = small.tile([16, 1], F32, tag="hs")
            nc.vector.tensor_tensor(out=hs[:], in0=h[:, 0:1], in1=h[:, 1:2],
                                    op=ALU.add)
            pca = pmp.tile([128, 2], F32, tag="misc")
            for t in range(2):
                nc.tensor.matmul(pca[:, t:t + 1],
                                 w2_sb[:, t * 128:(t + 1) * 128], hs[:],
                                 start=True, stop=True)
            ca = small.tile([128, 2], F32, tag="ca")
            nc.scalar.activation(ca[:], pca[:], ACTF.Sigmoid)
            ca_b = small.tile([128, 2], BF16, tag="ca_b")
            nc.scalar.activation(ca_b[:], pca[:], ACTF.Sigmoid)

            # ---------- xc0 on ACT; m1 = max(ca1*x1, xc0) fused STT ------
            xc0 = xcp.tile([128, HW], BF16, tag="xc0")
            nc.scalar.activation(xc0[:], xb[:, 0, :], ACTF.Copy,
                                 scale=ca[:, 0:1])
            # m1 layout [128, 3584]: column 64*y + x (x pad 56..63) so the
            # XBAR transpose below lands x on partitions, y in free dim.
            m1 = m1p.tile([128, MPAD], BF16, tag="m1")
            m1_v = m1[:].rearrange("p (y q) -> p y q", q=64)
            nc.gpsimd.memset(m1_v[:, :, H:64], 0.0)
            nc.vector.scalar_tensor_tensor(
                out=m1_v[:, :, 0:H],
                in0=xb[:, 1, :].rearrange("p (y x) -> p y x", x=W),
                scalar=ca[:, 1:2],
                in1=xc0[:].rearrange("p (y x) -> p y x", x=W),
                op0=ALU.mult, op1=ALU.max)

            # ---------- ssum via PE: 8 chunks of 392 at {0,32}x4 banks --
            # lhsT is the ca column broadcast to M=32 (stride-0): same
            # N-driven matmul cost, but fills pss completely so one drain
            # reads only initialized PSUM.
            pss = pssp.tile([64, 4, 512], F32, tag="pss")
            for t in range(2):
                ca_col = ca_b[:, t:t + 1]
                ca_m32 = bass.AP(
                    ca_col.tensor, ca_col.offset,
                    type(ca_col.ap)([list(ca_col.ap[0]), [0, 32]]))
                for j in range(8):
                    bp, bk = 32 * (j // 4), j % 4
                    nc.tensor.matmul(
                        pss[bp:bp + 32, bk, 0:SCHK],
                        ca_m32,
                        xb[:, t, j * SCHK:(j + 1) * SCHK],
                        start=(t == 0), stop=(t == 1),
                        skip_group_check=True)

            # ---------- smax: DMA XBAR transpose + DVE fold tree --------
            # HW XBAR semantic: out[a, b, c] = in[c, 128b + a] (verified on
            # device). With the 64y + x layout: mT[64h + x, q, c] =
            # pixel(y = 2q + h, x) of channel c. Rows x in 56..63 are pad.
            # NOTE: concurrent XBAR transposes corrupt each other (shared
            # ucode state) -- all transposes stay on one queue, one call.
            mT = mtp.tile([128, 28, 128], BF16, tag="mT")
            nc.sync.dma_start_transpose(mT[:, :, :], m1[:])
            sf1 = foldp.tile([128, 28, 64], BF16, tag="sf1")
            nc.vector.tensor_tensor(
                out=sf1[:], in0=mT[:, :, 0:64], in1=mT[:, :, 64:128],
                op=ALU.max)
            sf2 = foldp.tile([128, 28, 32], BF16, tag="sf2")
            nc.vector.tensor_tensor(
                out=sf2[:], in0=sf1[:, :, 0:32], in1=sf1[:, :, 32:64],
                op=ALU.max)
            sf3 = foldp.tile([128, 28, 16], BF16, tag="sf3")
            nc.vector.tensor_tensor(
                out=sf3[:], in0=sf2[:, :, 0:16], in1=sf2[:, :, 16:32],
                op=ALU.max)
            smT = smtp.tile([128, 28], BF16, tag="smT")
            nc.vector.tensor_reduce(
                out=smT[:], in_=sf3[:],
                axis=mybir.AxisListType.X, op=ALU.max)

            # ---------- conv input assembly ----------
            ssb = sap.tile([64, 4, SCHK], BF16, tag="ssb")
            nc.scalar.activation(ssb[:], pss[:, :, 0:SCHK], ACTF.Copy)
            # avg map, y-banded: rows y_in, cols x; chunk j=(p/32)*4+bank
            # holds image rows 7j..7j+6 (392 = 7*56)
            sp_avg = sap.tile([HP, WP], BF16, tag="sp_avg")
            nc.gpsimd.memset(sp_avg[:], 0.0)
            nc.gpsimd.dma_start(sp_avg[PAD:PAD + H, PAD:PAD + W],
                                ssb[0:33:32, :, :])
            # max map, TRANSPOSED (x-banded): rows x_in, cols y.
            # smT[64h + x, j] = maxpixel(y = 2j + h, x): even y from rows
            # 0..55, odd y from rows 64..119, stride-2 column interleave.
            sp_maxT = sap.tile([HP, WP], BF16, tag="sp_maxT")
            nc.gpsimd.memset(sp_maxT[:], 0.0)
            nc.scalar.dma_start(sp_maxT[PAD:PAD + H, PAD:PAD + H:2],
                                smT[0:H, :])
            nc.scalar.dma_start(sp_maxT[PAD:PAD + H, PAD + 1:PAD + H + 1:2],
                                smT[64:64 + H, :])

            # ---------- conv: avg y-banded + max x-banded + merge -------
            # max branch into its own PSUM bank, [x_out, y_out] layout
            pcvT = pmp.tile([H, W], F32, tag="misc")
            for dy in range(7):
                nc.tensor.matmul(pcvT[:], cb_sb[:, 1, dy, :],
                                 sp_maxT[:, dy:dy + W],
                                 start=(dy == 0), stop=(dy == 6))
            cvT_b = small.tile([H, W], BF16, tag="cvT_b")
            nc.scalar.activation(cvT_b[:], pcvT[:], ACTF.Copy)
            # avg branch accumulates in pcv; transposed max partial is
            # merged by a PE transpose (matmul vs identity) into the group
            pcv = pmp.tile([H, W], F32, tag="misc")
            for dx in range(7):
                nc.tensor.matmul(pcv[:], cb_sb[:, 0, dx, :],
                                 sp_avg[:, dx:dx + W],
                                 start=(dx == 0), stop=False)
            nc.tensor.matmul(pcv[:], cvT_b[:], ident[0:H, 0:W],
                             start=False, stop=True)
            sa_yx = small.tile([H, W], BF16, tag="sa_yx")
            nc.scalar.activation(sa_yx[:], pcv[:], ACTF.Sigmoid)

            # ---------- sa broadcast: GPSIMD ucode, two halves ----------
            sa_row = sap.tile([1, HW], BF16, tag="sa_row")
            nc.sync.dma_start(sa_row[:], sa_yx[:])
            sab = sabp.tile([128, HW], BF16, tag="sab")
            nc.gpsimd.partition_broadcast(sab[:, 0:HHW], sa_row[0:1, 0:HHW],
                                          channels=128)
            nc.gpsimd.partition_broadcast(sab[:, HHW:HW], sa_row[0:1, HHW:HW],
                                          channels=128)

            # ---------- final: ob = (x*ca)*sab via 4 quarter STTs ----
            ob = obp.tile([128, 2, HW], BF16, tag="ob")
            for t in range(2):
                for lo, hi in ((0, HHW), (HHW, HW)):
                    nc.vector.scalar_tensor_tensor(
                        out=ob[:, t, lo:hi], in0=xb[:, t, lo:hi],
                        scalar=ca[:, t:t + 1], in1=sab[:, lo:hi],
                        op0=ALU.mult, op1=ALU.mult)
                eng = nc.gpsimd if t == 0 else nc.scalar
                eng.dma_start(
                    out_ext[f, t * 128:(t + 1) * 128, :], ob[:, t, :])

    nc.finalize()
    return nc


_NC_CACHE = {}


def _get_nc(n_frames: int):
    if n_frames not in _NC_CACHE:
        _NC_CACHE[n_frames] = build_nc(n_frames)
    return _NC_CACHE[n_frames]


def _make_in_maps(f, w1, w2, conv_w):
    import ml_dtypes
    w1 = np.ascontiguousarray(np.asarray(w1, dtype=np.float32))
    w2 = np.ascontiguousarray(np.asarray(w2, dtype=np.float32))
    conv_w = np.asarray(conv_w, dtype=np.float32)
    convb = _build_conv_lhsT(conv_w)
    frames = np.asarray(f, dtype=np.float32).reshape(FRAMES, C, HW)
    frames = frames.astype(ml_dtypes.bfloat16)
    in_maps = []
    for i in range(N_CORES):
        in_maps.append({
            "x": np.ascontiguousarray(frames[i * FPC:(i + 1) * FPC]),
            "w1": w1,
            "w2": w2,
            "convb": convb,
        })
    return in_maps


def kernel(f: np.ndarray, w1: np.ndarray, w2: np.ndarray,
           conv_w: np.ndarray) -> np.ndarray:
    in_maps = _make_in_maps(f, w1, w2, conv_w)
    nc = _get_nc(FPC)
    res = run_bass_kernel_spmd(nc, in_maps, core_ids=list(range(N_CORES)))
    out = np.concatenate(
        [np.asarray(res.results[i]["out"]).astype(np.float32)
         for i in range(N_CORES)], axis=0)
    return out.reshape(B, T, C, H, W)


if __name__ == "__main__":
    rng = np.random.default_rng(0)
    f = rng.standard_normal((B, T, C, H, W), dtype=np.float32)
    w1 = rng.standard_normal((C, 16), dtype=np.float32) / 16.0
    w2 = rng.standard_normal((16, C), dtype=np.float32) / 4.0
    conv_w = rng.standard_normal((1, 2, 7, 7), dtype=np.float32) * 0.1
    out = kernel(f, w1, w2, conv_w)
    print("kernel ran, out shape", out.shape, out.dtype)


# revision 32
# speedup vs baseline: 1.0117x; 1.0117x over previous
"""CBAM (channel + spatial attention) Trainium2 kernel, 8-core data parallel.

Problem: f [8, 8, 256, 56, 56] f32 -> out same shape.
  x = f.reshape(BT, C, H, W)
  ca = sigmoid(mlp(max_hw(x)) + mlp(mean_hw(x)));  xc = ca * x
  s  = conv7x7([mean_c(xc); max_c(xc)]);           out = sigmoid(s) * xc

Strategy (per NeuronCore, 8 frames each, no collectives):
  - bf16 end-to-end on device: host casts f to bf16, device returns bf16,
    host upcasts -> halves HBM traffic (rel-err budget 2e-2 >> bf16 noise)
  - channel stats via DVE tensor_tensor_reduce: one pass per t computes
    (x_lo max x_hi) with fused max-accum -> pr_max; one TTR(add,add,
    scale=1/HW) -> pr_mean t0; t1 mean via ACT accum pass (engine balance)
  - m1 = max(ca1*x1, xc0) via fused scalar_tensor_tensor (xc1 never
    materialized); xc0 via ACT scaled copy
  - ssum: PE matmuls (ca_b^T x) into PSUM chunks at partitions {0,32,64}
    x 3 banks; GPS SWDGE scatters PSUM->conv rows with f32->bf16 cast
  - smax: m1 laid out [128, 3584] (28*128, pixel hw = 28p + j), DMA XBAR
    transpose (2 halves on SP/ACT queues) -> mT [128, 28, 128] SBUF bf16;
    DVE fold tree at 2x + small reduce -> smT [128, 28]; one affine
    scatter via (y h) j view = [64, 56]
  - conv 7x7 as 7 accumulating PE matmuls (banded lhsT, 1/C folded in)
  - sa broadcast via GPSIMD partition_broadcast in two halves
  - final: ob = (x*ca)*sab via 4 quarter scalar_tensor_tensor ops
  - stores: t0 on GPSIMD SWDGE, t1 on ACT HWDGE queue
"""

import sys
from contextlib import ExitStack

import numpy as np

if "/opt/trn_rl_repo" not in sys.path:
    sys.path.insert(0, "/opt/trn_rl_repo")

import concourse.bass as bass
import concourse.tile as tile
from concourse import bacc, mybir
from concourse.bass_utils import run_bass_kernel_spmd
from concourse.masks import make_identity

F32 = mybir.dt.float32
BF16 = mybir.dt.bfloat16
ALU = mybir.AluOpType
ACTF = mybir.ActivationFunctionType

N_CORES = 8
B, T, C, H, W = 8, 8, 256, 56, 56
HW = H * W            # 3136
FRAMES = B * T        # 64
FPC = FRAMES // N_CORES  # frames per core = 8
PAD = 3
HP, WP = H + 2 * PAD, W + 2 * PAD  # 62, 62
SCHK = 392            # ssum chunk width (8 chunks)
HHW = HW // 2         # 1568
MPAD = 28 * 128       # 3584: m1 padded so hw = 28*p + j transposes cleanly
NEG = -3.0e38


def _build_conv_lhsT(conv_w: np.ndarray) -> np.ndarray:
    """Banded matrices for the 7x7 conv as accumulating matmuls.

    Branch 0 (avg, y-banded, 1/C folded): B[0, dx][yi, yo] =
    w_eff[0, yi-yo, dx] -- contracts y_in for each of 7 dx columns.
    Branch 1 (max, x-banded, operates on the transposed max map):
    B[1, dy][xi, xo] = w_eff[1, dy, xi-xo] -- contracts x_in for each
    of 7 dy rows; its [x_out, y_out] result is transposed back on PE.
    """
    w_eff = conv_w[0].astype(np.float64).copy()  # [2, 7, 7]
    w_eff[0] /= C
    Bm = np.zeros((2, 7, HP, H), dtype=np.float32)
    di = np.arange(7)
    for d in range(7):
        for o in range(H):
            Bm[0, d, o + di, o] = w_eff[0, :, d]   # dx = d, band over y
            Bm[1, d, o + di, o] = w_eff[1, d, :]   # dy = d, band over x
    return Bm


def build_nc(n_frames: int = FPC):
    nc = bacc.Bacc("TRN2", target_bir_lowering=False, debug=False,
                   num_devices=N_CORES)

    x_ext = nc.dram_tensor("x", [n_frames, C, HW], BF16, kind="ExternalInput")
    w1_ext = nc.dram_tensor("w1", [C, 16], F32, kind="ExternalInput")
    w2_ext = nc.dram_tensor("w2", [16, C], F32, kind="ExternalInput")
    cb_ext = nc.dram_tensor("convb", [2, 7, HP, H], F32, kind="ExternalInput")
    out_ext = nc.dram_tensor("out", [n_frames, C, HW], BF16,
                             kind="ExternalOutput")

    with tile.TileContext(nc) as tc, ExitStack() as ctx:
        consts = ctx.enter_context(tc.tile_pool(name="consts", bufs=1))
        xin = ctx.enter_context(tc.tile_pool(name="xin", bufs=3))
        scrp = ctx.enter_context(tc.tile_pool(name="scr", bufs=1))
        xcp = ctx.enter_context(tc.tile_pool(name="xc", bufs=2))
        m1p = ctx.enter_context(tc.tile_pool(name="m1", bufs=2))
        mtp = ctx.enter_context(tc.tile_pool(name="mt", bufs=2))
        foldp = ctx.enter_context(tc.tile_pool(name="fold", bufs=1))
        smtp = ctx.enter_context(tc.tile_pool(name="smt", bufs=2))
        sabp = ctx.enter_context(tc.tile_pool(name="sab", bufs=2))
        obp = ctx.enter_context(tc.tile_pool(name="ob", bufs=2))
        sap = ctx.enter_context(tc.tile_pool(name="sa", bufs=2))
        small = ctx.enter_context(tc.tile_pool(name="small", bufs=3))
        # PSUM: pss 4 banks + misc 2 banks = 6
        pssp = ctx.enter_context(tc.tile_pool(name="pss", bufs=1, space="PSUM"))
        pmp = ctx.enter_context(tc.tile_pool(name="pm", bufs=2, space="PSUM"))

        # ---- constants / weights (loaded once) ----
        w1_sb = consts.tile([128, 2, 16], F32)       # [k, ktile, m]
        for t in range(2):
            nc.sync.dma_start(w1_sb[:, t, :], w1_ext[t * 128:(t + 1) * 128, :])
        w2_sb = consts.tile([16, C], F32)
        nc.sync.dma_start(w2_sb[:], w2_ext[:, :])
        cb_sb = consts.tile([HP, 2, 7, H], BF16)     # [in_row, branch, d, out]
        nc.gpsimd.dma_start(                          # SWDGE: casts f32->bf16
            cb_sb[:],
            cb_ext.rearrange("b d p y -> p b d y"),
        )
        ident = consts.tile([64, 64], BF16)
        make_identity(nc, ident[:])

        xbs = {}

        def load_frame(g):
            if g >= n_frames:
                return
            xb = xin.tile([128, 2, HW], BF16, tag="x")
            for t in range(2):
                nc.sync.dma_start(
                    xb[:, t, :], x_ext[g, t * 128:(t + 1) * 128, :])
            xbs[g] = xb

        load_frame(0)
        load_frame(1)

        for f in range(n_frames):
            load_frame(f + 2)
            xb = xbs.pop(f)

            # ---------- stats: max fold tree (DVE); means via accumulate
            # (t0 on DVE tensor_scalar f32-accum, t1 on ACT -- balance) ---
            pr_max = small.tile([128, 2], F32, tag="prmax")
            pr_sum = small.tile([128, 2], F32, tag="prsum")
            f1 = foldp.tile([128, 2, HHW], BF16, tag="f1")
            nc.vector.tensor_tensor(
                out=f1[:], in0=xb[:, :, 0:HHW], in1=xb[:, :, HHW:HW],
                op=ALU.max)
            f2 = foldp.tile([128, 2, 784], BF16, tag="f2")
            nc.vector.tensor_tensor(
                out=f2[:], in0=f1[:, :, 0:784], in1=f1[:, :, 784:HHW],
                op=ALU.max)
            f3 = foldp.tile([128, 2, 392], BF16, tag="f3")
            nc.vector.tensor_tensor(
                out=f3[:], in0=f2[:, :, 0:392], in1=f2[:, :, 392:784],
                op=ALU.max)
            nc.vector.tensor_reduce(
                out=pr_max[:], in_=f3[:],
                axis=mybir.AxisListType.X, op=ALU.max)
            scr1 = scrp.tile([128, 2, HW], BF16, tag="scr1")
            for t in range(2):
                nc.scalar.activation(
                    scr1[:, t, :], xb[:, t, :], ACTF.Copy, scale=1.0 / HW,
                    accum_out=pr_sum[:, t:t + 1])

            # ---------- MLP on PE (stats are already mean-scaled) --------
            ph = pmp.tile([16, 2], F32, tag="misc")
            for si, prs in ((0, pr_max), (1, pr_sum)):
                for t in range(2):
                    nc.tensor.matmul(ph[:, si:si + 1], w1_sb[:, t, :],
                                     prs[:, t:t + 1],
                                     start=(t == 0), stop=(t == 1))
            h = small.tile([16, 2], F32, tag="h")
            nc.scalar.activation(h[:], ph[:], ACTF.Relu)
            hs = small.tile([16, 1], F32, tag="hs")
            nc.vector.tensor_tensor(out=hs[:], in0=h[:, 0:1], in1=h[:, 1:2],
                                    op=ALU.add)
            pca = pmp.tile([128, 2], F32, tag="misc")
            for t in range(2):
                nc.tensor.matmul(pca[:, t:t + 1],
                                 w2_sb[:, t * 128:(t + 1) * 128], hs[:],
                                 start=True, stop=True)
            ca = small.tile([128, 2], F32, tag="ca")
            nc.scalar.activation(ca[:], pca[:], ACTF.Sigmoid)
            ca_b = small.tile([128, 2], BF16, tag="ca_b")
            nc.scalar.activation(ca_b[:], pca[:], ACTF.Sigmoid)

            # ---------- xc0 on ACT, xc1 on DVE; m1 = max(xc0, xc1) -------
            xc0 = xcp.tile([128, HW], BF16, tag="xc0")
            nc.scalar.activation(xc0[:], xb[:, 0, :], ACTF.Copy,
                                 scale=ca[:, 0:1])
            xc1 = xcp.tile([128, HW], BF16, tag="xc1")
            nc.vector.tensor_scalar(
                out=xc1[:], in0=xb[:, 1, :],
                scalar1=ca[:, 1:2], scalar2=None, op0=ALU.mult)
            # m1 layout [128, 3584]: column 64*y + x (x pad 56..63) so the
            # XBAR transpose below lands x on partitions, y in free dim.
            m1 = m1p.tile([128, MPAD], BF16, tag="m1")
            m1_v = m1[:].rearrange("p (y q) -> p y q", q=64)
            nc.gpsimd.memset(m1_v[:, :, H:64], 0.0)
            nc.vector.tensor_tensor(
                out=m1_v[:, :, 0:H],
                in0=xc0[:].rearrange("p (y x) -> p y x", x=W),
                in1=xc1[:].rearrange("p (y x) -> p y x", x=W),
                op=ALU.max)

            # ---------- ssum via PE: 8 chunks of 392 at {0,32}x4 banks --
            # lhsT is the ca column broadcast to M=32 (stride-0): same
            # N-driven matmul cost, but fills pss completely so one drain
            # reads only initialized PSUM.
            pss = pssp.tile([64, 4, 512], F32, tag="pss")
            for t in range(2):
                ca_col = ca_b[:, t:t + 1]
                ca_m32 = bass.AP(
                    ca_col.tensor, ca_col.offset,
                    type(ca_col.ap)([list(ca_col.ap[0]), [0, 32]]))
                for j in range(8):
                    bp, bk = 32 * (j // 4), j % 4
                    nc.tensor.matmul(
                        pss[bp:bp + 32, bk, 0:SCHK],
                        ca_m32,
                        xb[:, t, j * SCHK:(j + 1) * SCHK],
                        start=(t == 0), stop=(t == 1),
                        skip_group_check=True)

            # ---------- smax: DMA XBAR transpose + DVE fold tree --------
            # HW XBAR semantic: out[a, b, c] = in[c, 128b + a] (verified on
            # device). With the 64y + x layout: mT[64h + x, q, c] =
            # pixel(y = 2q + h, x) of channel c. Rows x in 56..63 are pad.
            # NOTE: concurrent XBAR transposes corrupt each other (shared
            # ucode state) -- all transposes stay on one queue, one call.
            mT = mtp.tile([128, 28, 128], BF16, tag="mT")
            nc.scalar.dma_start_transpose(mT[:, :, :], m1[:])
            sf1 = foldp.tile([128, 28, 64], BF16, tag="sf1")
            nc.vector.tensor_tensor(
                out=sf1[:], in0=mT[:, :, 0:64], in1=mT[:, :, 64:128],
                op=ALU.max)
            sf2 = foldp.tile([128, 28, 32], BF16, tag="sf2")
            nc.vector.tensor_tensor(
                out=sf2[:], in0=sf1[:, :, 0:32], in1=sf1[:, :, 32:64],
                op=ALU.max)
            sf3 = foldp.tile([128, 28, 16], BF16, tag="sf3")
            nc.vector.tensor_tensor(
                out=sf3[:], in0=sf2[:, :, 0:16], in1=sf2[:, :, 16:32],
                op=ALU.max)
            smT = smtp.tile([128, 28], BF16, tag="smT")
            nc.vector.tensor_reduce(
                out=smT[:], in_=sf3[:],
                axis=mybir.AxisListType.X, op=ALU.max)

            # ---------- conv input assembly ----------
            ssb = sap.tile([64, 4, SCHK], BF16, tag="ssb")
            nc.scalar.activation(ssb[:], pss[:, :, 0:SCHK], ACTF.Copy)
            # avg map, y-banded: rows y_in, cols x; chunk j=(p/32)*4+bank
            # holds image rows 7j..7j+6 (392 = 7*56)
            sp_avg = sap.tile([HP, WP], BF16, tag="sp_avg")
            nc.gpsimd.memset(sp_avg[:], 0.0)
            nc.gpsimd.dma_start(sp_avg[PAD:PAD + H, PAD:PAD + W],
                                ssb[0:33:32, :, :])
            # max map, TRANSPOSED (x-banded): rows x_in, cols y.
            # smT[64h + x, j] = maxpixel(y = 2j + h, x): even y from rows
            # 0..55, odd y from rows 64..119, stride-2 column interleave.
            sp_maxT = sap.tile([HP, WP], BF16, tag="sp_maxT")
            nc.gpsimd.memset(sp_maxT[:], 0.0)
            nc.gpsimd.dma_start(sp_maxT[PAD:PAD + H, PAD:PAD + H:2],
                                smT[0:H, :])
            nc.gpsimd.dma_start(sp_maxT[PAD:PAD + H, PAD + 1:PAD + H + 1:2],
                                smT[64:64 + H, :])

            # ---------- conv: avg y-banded + max x-banded + merge -------
            # max branch into its own PSUM bank, [x_out, y_out] layout
            pcvT = pmp.tile([H, W], F32, tag="misc")
            for dy in range(7):
                nc.tensor.matmul(pcvT[:], cb_sb[:, 1, dy, :],
                                 sp_maxT[:, dy:dy + W],
                                 start=(dy == 0), stop=(dy == 6))
            cvT_b = small.tile([H, W], BF16, tag="cvT_b")
            nc.scalar.activation(cvT_b[:], pcvT[:], ACTF.Copy)
            # avg branch accumulates in pcv; transposed max partial is
            # merged by a PE transpose (matmul vs identity) into the group
            pcv = pmp.tile([H, W], F32, tag="misc")
            for dx in range(7):
                nc.tensor.matmul(pcv[:], cb_sb[:, 0, dx, :],
                                 sp_avg[:, dx:dx + W],
                                 start=(dx == 0), stop=False)
            nc.tensor.matmul(pcv[:], cvT_b[:], ident[0:H, 0:W],
                             start=False, stop=True)
            sa_yx = small.tile([H, W], BF16, tag="sa_yx")
            nc.scalar.activation(sa_yx[:], pcv[:], ACTF.Sigmoid)

            # ---------- sa broadcast: GPSIMD ucode ----------
            sa_row = sap.tile([1, HW], BF16, tag="sa_row")
            nc.sync.dma_start(sa_row[:], sa_yx[:])
            sab = sabp.tile([128, HW], BF16, tag="sab")
            nc.gpsimd.partition_broadcast(sab[:], sa_row[0:1, :],
                                          channels=128)

            # ---------- final: ob_t = xc_t * sab (TT), stores ----
            ob = obp.tile([128, 2, HW], BF16, tag="ob")
            for t, xc in ((0, xc0), (1, xc1)):
                nc.vector.tensor_tensor(
                    out=ob[:, t, :], in0=xc[:], in1=sab[:], op=ALU.mult)
                eng = nc.gpsimd if t == 0 else nc.sync
                eng.dma_start(
                    out_ext[f, t * 128:(t + 1) * 128, :], ob[:, t, :])

    nc.finalize()
    return nc


_NC_CACHE = {}


def _get_nc(n_frames: int):
    if n_frames not in _NC_CACHE:
        _NC_CACHE[n_frames] = build_nc(n_frames)
    return _NC_CACHE[n_frames]


def _make_in_maps(f, w1, w2, conv_w):
    import ml_dtypes
    w1 = np.ascontiguousarray(np.asarray(w1, dtype=np.float32))
    w2 = np.ascontiguousarray(np.asarray(w2, dtype=np.float32))
    conv_w = np.asarray(conv_w, dtype=np.float32)
    convb = _build_conv_lhsT(conv_w)
    frames = np.asarray(f, dtype=np.float32).reshape(FRAMES, C, HW)
    frames = frames.astype(ml_dtypes.bfloat16)
    in_maps = []
    for i in range(N_CORES):
        in_maps.append({
            "x": np.ascontiguousarray(frames[i * FPC:(i + 1) * FPC]),
            "w1": w1,
            "w2": w2,
            "convb": convb,
        })
    return in_maps


def kernel(f: np.ndarray, w1: np.ndarray, w2: np.ndarray,
           conv_w: np.ndarray) -> np.ndarray:
    in_maps = _make_in_maps(f, w1, w2, conv_w)
    nc = _get_nc(FPC)
    res = run_bass_kernel_spmd(nc, in_maps, core_ids=list(range(N_CORES)))
    out = np.concatenate(
        [np.asarray(res.results[i]["out"]).astype(np.float32)
         for i in range(N_CORES)], axis=0)
    return out.reshape(B, T, C, H, W)


if __name__ == "__main__":
    rng = np.random.default_rng(0)
    f = rng.standard_normal((B, T, C, H, W), dtype=np.float32)
    w1 = rng.standard_normal((C, 16), dtype=np.float32) / 16.0
    w2 = rng.standard_normal((16, C), dtype=np.float32) / 4.0
    conv_w = rng.standard_normal((1, 2, 7, 7), dtype=np.float32) * 0.1
    out = kernel(f, w1, w2, conv_w)
    print("kernel ran, out shape", out.shape, out.dtype)


# revision 35
# speedup vs baseline: 1.2493x; 1.2349x over previous
"""CBAM (channel + spatial attention) Trainium2 kernel, 8-core data parallel.

Problem: f [8, 8, 256, 56, 56] f32 -> out same shape.
  x = f.reshape(BT, C, H, W)
  ca = sigmoid(mlp(max_hw(x)) + mlp(mean_hw(x)));  xc = ca * x
  s  = conv7x7([mean_c(xc); max_c(xc)]);           out = sigmoid(s) * xc

Strategy (per NeuronCore, 8 frames each, no collectives):
  - bf16 end-to-end on device: host casts f to bf16, device returns bf16,
    host upcasts -> halves HBM traffic (rel-err budget 2e-2 >> bf16 noise)
  - channel stats via DVE tensor_tensor_reduce: one pass per t computes
    (x_lo max x_hi) with fused max-accum -> pr_max; one TTR(add,add,
    scale=1/HW) -> pr_mean t0; t1 mean via ACT accum pass (engine balance)
  - m1 = max(ca1*x1, xc0) via fused scalar_tensor_tensor (xc1 never
    materialized); xc0 via ACT scaled copy
  - ssum: PE matmuls (ca_b^T x) into PSUM chunks at partitions {0,32,64}
    x 3 banks; GPS SWDGE scatters PSUM->conv rows with f32->bf16 cast
  - smax: m1 laid out [128, 3584] (28*128, pixel hw = 28p + j), DMA XBAR
    transpose (2 halves on SP/ACT queues) -> mT [128, 28, 128] SBUF bf16;
    DVE fold tree at 2x + small reduce -> smT [128, 28]; one affine
    scatter via (y h) j view = [64, 56]
  - conv 7x7 as 7 accumulating PE matmuls (banded lhsT, 1/C folded in)
  - sa broadcast via GPSIMD partition_broadcast in two halves
  - final: ob = (x*ca)*sab via 4 quarter scalar_tensor_tensor ops
  - stores: t0 on GPSIMD SWDGE, t1 on ACT HWDGE queue
"""

import sys
from contextlib import ExitStack

import numpy as np

if "/opt/trn_rl_repo" not in sys.path:
    sys.path.insert(0, "/opt/trn_rl_repo")

import concourse.bass as bass
import concourse.tile as tile
from concourse import bacc, mybir
from concourse.bass_utils import run_bass_kernel_spmd
from concourse.masks import make_identity

F32 = mybir.dt.float32
BF16 = mybir.dt.bfloat16
ALU = mybir.AluOpType
ACTF = mybir.ActivationFunctionType

N_CORES = 8
B, T, C, H, W = 8, 8, 256, 56, 56
HW = H * W            # 3136
FRAMES = B * T        # 64
FPC = FRAMES // N_CORES  # frames per core = 8
PAD = 3
HP, WP = H + 2 * PAD, W + 2 * PAD  # 62, 62
SCHK = 392            # ssum chunk width (8 chunks)
HHW = HW // 2         # 1568
MPAD = 28 * 128       # 3584: m1 padded so hw = 28*p + j transposes cleanly
NEG = -3.0e38


def _build_conv_lhsT(conv_w: np.ndarray) -> np.ndarray:
    """Banded matrices for the 7x7 conv as accumulating matmuls.

    Branch 0 (avg, y-banded, 1/C folded): B[0, dx][yi, yo] =
    w_eff[0, yi-yo, dx] -- contracts y_in for each of 7 dx columns.
    Branch 1 (max, x-banded, operates on the transposed max map):
    B[1, dy][xi, xo] = w_eff[1, dy, xi-xo] -- contracts x_in for each
    of 7 dy rows; its [x_out, y_out] result is transposed back on PE.
    """
    w_eff = conv_w[0].astype(np.float64).copy()  # [2, 7, 7]
    w_eff[0] /= C
    Bm = np.zeros((2, 7, HP, H), dtype=np.float32)
    di = np.arange(7)
    for d in range(7):
        for o in range(H):
            Bm[0, d, o + di, o] = w_eff[0, :, d]   # dx = d, band over y
            Bm[1, d, o + di, o] = w_eff[1, d, :]   # dy = d, band over x
    return Bm


def build_nc(n_frames: int = FPC):
    nc = bacc.Bacc("TRN2", target_bir_lowering=False, debug=False,
                   num_devices=N_CORES)

    x_ext = nc.dram_tensor("x", [n_frames, C, HW], BF16, kind="ExternalInput")
    w1_ext = nc.dram_tensor("w1", [C, 16], F32, kind="ExternalInput")
    w2_ext = nc.dram_tensor("w2", [16, C], F32, kind="ExternalInput")
    cb_ext = nc.dram_tensor("convb", [2, 7, HP, H], F32, kind="ExternalInput")
    out_ext = nc.dram_tensor("out", [n_frames, C, HW], BF16,
                             kind="ExternalOutput")

    with tile.TileContext(nc) as tc, ExitStack() as ctx:
        consts = ctx.enter_context(tc.tile_pool(name="consts", bufs=1))
        xin = ctx.enter_context(tc.tile_pool(name="xin", bufs=3))
        scrp = ctx.enter_context(tc.tile_pool(name="scr", bufs=1))
        xcp = ctx.enter_context(tc.tile_pool(name="xc", bufs=2))
        m1p = ctx.enter_context(tc.tile_pool(name="m1", bufs=2))
        mtp = ctx.enter_context(tc.tile_pool(name="mt", bufs=2))
        foldp = ctx.enter_context(tc.tile_pool(name="fold", bufs=1))
        smtp = ctx.enter_context(tc.tile_pool(name="smt", bufs=2))
        sabp = ctx.enter_context(tc.tile_pool(name="sab", bufs=2))
        obp = ctx.enter_context(tc.tile_pool(name="ob", bufs=2))
        sap = ctx.enter_context(tc.tile_pool(name="sa", bufs=2))
        small = ctx.enter_context(tc.tile_pool(name="small", bufs=3))
        # PSUM: pss 4 banks + misc 2 banks = 6
        pssp = ctx.enter_context(tc.tile_pool(name="pss", bufs=1, space="PSUM"))
        pmp = ctx.enter_context(tc.tile_pool(name="pm", bufs=2, space="PSUM"))

        # ---- constants / weights (loaded once) ----
        w1_sb = consts.tile([128, 2, 16], F32)       # [k, ktile, m]
        for t in range(2):
            nc.sync.dma_start(w1_sb[:, t, :], w1_ext[t * 128:(t + 1) * 128, :])
        w2_sb = consts.tile([16, C], F32)
        nc.sync.dma_start(w2_sb[:], w2_ext[:, :])
        cb_sb = consts.tile([HP, 2, 7, H], BF16)     # [in_row, branch, d, out]
        nc.gpsimd.dma_start(                          # SWDGE: casts f32->bf16
            cb_sb[:],
            cb_ext.rearrange("b d p y -> p b d y"),
        )
        ident = consts.tile([64, 64], BF16)
        make_identity(nc, ident[:])

        xbs = {}

        def load_frame(g):
            if g >= n_frames:
                return
            xb = xin.tile([128, 2, HW], BF16, tag="x")
            for t in range(2):
                nc.sync.dma_start(
                    xb[:, t, :], x_ext[g, t * 128:(t + 1) * 128, :])
            xbs[g] = xb

        def emit_ob(state):
            if state is None:
                return
            g, g_xc0, g_xc1, g_sab = state
            ob = obp.tile([128, 2, HW], BF16, tag="ob")
            for t, xc in ((0, g_xc0), (1, g_xc1)):
                nc.vector.tensor_tensor(
                    out=ob[:, t, :], in0=xc[:], in1=g_sab[:], op=ALU.mult)
                eng = nc.gpsimd if t == 0 else nc.sync
                eng.dma_start(
                    out_ext[g, t * 128:(t + 1) * 128, :], ob[:, t, :])

        load_frame(0)
        load_frame(1)
        prev = None

        for f in range(n_frames):
            load_frame(f + 2)
            xb = xbs.pop(f)

            # ---------- stats: max fold tree (DVE); means via accumulate
            # (t0 on DVE tensor_scalar f32-accum, t1 on ACT -- balance) ---
            pr_max = small.tile([128, 2], F32, tag="prmax")
            pr_sum = small.tile([128, 2], F32, tag="prsum")
            f1 = foldp.tile([128, 2, HHW], BF16, tag="f1")
            nc.vector.tensor_tensor(
                out=f1[:], in0=xb[:, :, 0:HHW], in1=xb[:, :, HHW:HW],
                op=ALU.max)
            f2 = foldp.tile([128, 2, 784], BF16, tag="f2")
            nc.vector.tensor_tensor(
                out=f2[:], in0=f1[:, :, 0:784], in1=f1[:, :, 784:HHW],
                op=ALU.max)
            f3 = foldp.tile([128, 2, 392], BF16, tag="f3")
            nc.vector.tensor_tensor(
                out=f3[:], in0=f2[:, :, 0:392], in1=f2[:, :, 392:784],
                op=ALU.max)
            nc.vector.tensor_reduce(
                out=pr_max[:], in_=f3[:],
                axis=mybir.AxisListType.X, op=ALU.max)
            scr1 = scrp.tile([128, 2, HW], BF16, tag="scr1")
            for t in range(2):
                nc.scalar.activation(
                    scr1[:, t, :], xb[:, t, :], ACTF.Copy, scale=1.0 / HW,
                    accum_out=pr_sum[:, t:t + 1])

            # ---------- MLP on PE (stats are already mean-scaled) --------
            ph = pmp.tile([16, 2], F32, tag="misc")
            for si, prs in ((0, pr_max), (1, pr_sum)):
                for t in range(2):
                    nc.tensor.matmul(ph[:, si:si + 1], w1_sb[:, t, :],
                                     prs[:, t:t + 1],
                                     start=(t == 0), stop=(t == 1))
            h = small.tile([16, 2], F32, tag="h")
            nc.scalar.activation(h[:], ph[:], ACTF.Relu)
            hs = small.tile([16, 1], F32, tag="hs")
            nc.vector.tensor_tensor(out=hs[:], in0=h[:, 0:1], in1=h[:, 1:2],
                                    op=ALU.add)
            pca = pmp.tile([128, 2], F32, tag="misc")
            for t in range(2):
                nc.tensor.matmul(pca[:, t:t + 1],
                                 w2_sb[:, t * 128:(t + 1) * 128], hs[:],
                                 start=True, stop=True)
            ca = small.tile([128, 2], F32, tag="ca")
            nc.scalar.activation(ca[:], pca[:], ACTF.Sigmoid)
            ca_b = small.tile([128, 2], BF16, tag="ca_b")
            nc.scalar.activation(ca_b[:], pca[:], ACTF.Sigmoid)

            # ---------- xc0 on ACT, xc1 on DVE; m1 = max(xc0, xc1) -------
            xc0 = xcp.tile([128, HW], BF16, tag="xc0")
            nc.scalar.activation(xc0[:], xb[:, 0, :], ACTF.Copy,
                                 scale=ca[:, 0:1])
            xc1 = xcp.tile([128, HW], BF16, tag="xc1")
            nc.vector.tensor_scalar(
                out=xc1[:], in0=xb[:, 1, :],
                scalar1=ca[:, 1:2], scalar2=None, op0=ALU.mult)
            # m1 layout [128, 3584]: column 64*y + x (x pad 56..63) so the
            # XBAR transpose below lands x on partitions, y in free dim.
            m1 = m1p.tile([128, MPAD], BF16, tag="m1")
            m1_v = m1[:].rearrange("p (y q) -> p y q", q=64)
            nc.gpsimd.memset(m1_v[:, :, H:64], 0.0)
            nc.vector.tensor_tensor(
                out=m1_v[:, :, 0:H],
                in0=xc0[:].rearrange("p (y x) -> p y x", x=W),
                in1=xc1[:].rearrange("p (y x) -> p y x", x=W),
                op=ALU.max)

            # ---------- ssum via PE: 8 chunks of 392 at {0,32}x4 banks --
            # lhsT is the ca column broadcast to M=32 (stride-0): same
            # N-driven matmul cost, but fills pss completely so one drain
            # reads only initialized PSUM.
            pss = pssp.tile([64, 4, 512], F32, tag="pss")
            for t in range(2):
                ca_col = ca_b[:, t:t + 1]
                ca_m32 = bass.AP(
                    ca_col.tensor, ca_col.offset,
                    type(ca_col.ap)([list(ca_col.ap[0]), [0, 32]]))
                for j in range(8):
                    bp, bk = 32 * (j // 4), j % 4
                    nc.tensor.matmul(
                        pss[bp:bp + 32, bk, 0:SCHK],
                        ca_m32,
                        xb[:, t, j * SCHK:(j + 1) * SCHK],
                        start=(t == 0), stop=(t == 1),
                        skip_group_check=True)

            # ---------- smax: DMA XBAR transpose + DVE fold tree --------
            # HW XBAR semantic: out[a, b, c] = in[c, 128b + a] (verified on
            # device). With the 64y + x layout: mT[64h + x, q, c] =
            # pixel(y = 2q + h, x) of channel c. Rows x in 56..63 are pad.
            # NOTE: concurrent XBAR transposes corrupt each other (shared
            # ucode state) -- all transposes stay on one queue, one call.
            mT = mtp.tile([128, 28, 128], BF16, tag="mT")
            nc.sync.dma_start_transpose(mT[:, :, :], m1[:])

            # ---------- previous frame's final multiply + stores --------
            # (software pipelining: fills the XBAR-transpose latency gap
            # so DVE isn't head-of-line blocked on the sa chain)
            emit_ob(prev)
            prev = None
            sf1 = foldp.tile([128, 28, 64], BF16, tag="sf1")
            nc.vector.tensor_tensor(
                out=sf1[:], in0=mT[:, :, 0:64], in1=mT[:, :, 64:128],
                op=ALU.max)
            sf2 = foldp.tile([128, 28, 32], BF16, tag="sf2")
            nc.vector.tensor_tensor(
                out=sf2[:], in0=sf1[:, :, 0:32], in1=sf1[:, :, 32:64],
                op=ALU.max)
            sf3 = foldp.tile([128, 28, 16], BF16, tag="sf3")
            nc.vector.tensor_tensor(
                out=sf3[:], in0=sf2[:, :, 0:16], in1=sf2[:, :, 16:32],
                op=ALU.max)
            smT = smtp.tile([128, 28], BF16, tag="smT")
            nc.vector.tensor_reduce(
                out=smT[:], in_=sf3[:],
                axis=mybir.AxisListType.X, op=ALU.max)

            # ---------- conv input assembly ----------
            ssb = sap.tile([64, 4, SCHK], BF16, tag="ssb")
            nc.scalar.activation(ssb[:], pss[:, :, 0:SCHK], ACTF.Copy)
            # avg map, y-banded: rows y_in, cols x; chunk j=(p/32)*4+bank
            # holds image rows 7j..7j+6 (392 = 7*56)
            sp_avg = sap.tile([HP, WP], BF16, tag="sp_avg")
            nc.gpsimd.memset(sp_avg[:], 0.0)
            nc.gpsimd.dma_start(sp_avg[PAD:PAD + H, PAD:PAD + W],
                                ssb[0:33:32, :, :])
            # max map, TRANSPOSED (x-banded): rows x_in, cols y.
            # smT[64h + x, j] = maxpixel(y = 2j + h, x): even y from rows
            # 0..55, odd y from rows 64..119, stride-2 column interleave.
            sp_maxT = sap.tile([HP, WP], BF16, tag="sp_maxT")
            nc.gpsimd.memset(sp_maxT[:], 0.0)
            nc.gpsimd.dma_start(sp_maxT[PAD:PAD + H, PAD:PAD + H:2],
                                smT[0:H, :])
            nc.gpsimd.dma_start(sp_maxT[PAD:PAD + H, PAD + 1:PAD + H + 1:2],
                                smT[64:64 + H, :])

            # ---------- conv: avg y-banded + max x-banded + merge -------
            # max branch into its own PSUM bank, [x_out, y_out] layout
            pcvT = pmp.tile([H, W], F32, tag="misc")
            for dy in range(7):
                nc.tensor.matmul(pcvT[:], cb_sb[:, 1, dy, :],
                                 sp_maxT[:, dy:dy + W],
                                 start=(dy == 0), stop=(dy == 6))
            cvT_b = small.tile([H, W], BF16, tag="cvT_b")
            nc.scalar.activation(cvT_b[:], pcvT[:], ACTF.Copy)
            # avg branch accumulates in pcv; transposed max partial is
            # merged by a PE transpose (matmul vs identity) into the group
            pcv = pmp.tile([H, W], F32, tag="misc")
            for dx in range(7):
                nc.tensor.matmul(pcv[:], cb_sb[:, 0, dx, :],
                                 sp_avg[:, dx:dx + W],
                                 start=(dx == 0), stop=False)
            nc.tensor.matmul(pcv[:], cvT_b[:], ident[0:H, 0:W],
                             start=False, stop=True)
            sa_yx = small.tile([H, W], BF16, tag="sa_yx")
            nc.scalar.activation(sa_yx[:], pcv[:], ACTF.Sigmoid)

            # ---------- sa broadcast: GPSIMD ucode ----------
            sa_row = sap.tile([1, HW], BF16, tag="sa_row")
            nc.scalar.dma_start(sa_row[:], sa_yx[:])
            sab = sabp.tile([128, HW], BF16, tag="sab")
            nc.gpsimd.partition_broadcast(sab[:], sa_row[0:1, :],
                                          channels=128)
            prev = (f, xc0, xc1, sab)

        emit_ob(prev)

    nc.finalize()
    return nc


_NC_CACHE = {}


def _get_nc(n_frames: int):
    if n_frames not in _NC_CACHE:
        _NC_CACHE[n_frames] = build_nc(n_frames)
    return _NC_CACHE[n_frames]


def _make_in_maps(f, w1, w2, conv_w):
    import ml_dtypes
    w1 = np.ascontiguousarray(np.asarray(w1, dtype=np.float32))
    w2 = np.ascontiguousarray(np.asarray(w2, dtype=np.float32))
    conv_w = np.asarray(conv_w, dtype=np.float32)
    convb = _build_conv_lhsT(conv_w)
    frames = np.asarray(f, dtype=np.float32).reshape(FRAMES, C, HW)
    frames = frames.astype(ml_dtypes.bfloat16)
    in_maps = []
    for i in range(N_CORES):
        in_maps.append({
            "x": np.ascontiguousarray(frames[i * FPC:(i + 1) * FPC]),
            "w1": w1,
            "w2": w2,
            "convb": convb,
        })
    return in_maps


def kernel(f: np.ndarray, w1: np.ndarray, w2: np.ndarray,
           conv_w: np.ndarray) -> np.ndarray:
    in_maps = _make_in_maps(f, w1, w2, conv_w)
    nc = _get_nc(FPC)
    res = run_bass_kernel_spmd(nc, in_maps, core_ids=list(range(N_CORES)))
    out = np.concatenate(
        [np.asarray(res.results[i]["out"]).astype(np.float32)
         for i in range(N_CORES)], axis=0)
    return out.reshape(B, T, C, H, W)


if __name__ == "__main__":
    rng = np.random.default_rng(0)
    f = rng.standard_normal((B, T, C, H, W), dtype=np.float32)
    w1 = rng.standard_normal((C, 16), dtype=np.float32) / 16.0
    w2 = rng.standard_normal((16, C), dtype=np.float32) / 4.0
    conv_w = rng.standard_normal((1, 2, 7, 7), dtype=np.float32) * 0.1
    out = kernel(f, w1, w2, conv_w)
    print("kernel ran, out shape", out.shape, out.dtype)


# revision 36
# speedup vs baseline: 1.4234x; 1.1393x over previous
"""CBAM (channel + spatial attention) Trainium2 kernel, 8-core data parallel.

Problem: f [8, 8, 256, 56, 56] f32 -> out same shape.
  x = f.reshape(BT, C, H, W)
  ca = sigmoid(mlp(max_hw(x)) + mlp(mean_hw(x)));  xc = ca * x
  s  = conv7x7([mean_c(xc); max_c(xc)]);           out = sigmoid(s) * xc

Strategy (per NeuronCore, 8 frames each, no collectives):
  - bf16 end-to-end on device: host casts f to bf16, device returns bf16,
    host upcasts -> halves HBM traffic (rel-err budget 2e-2 >> bf16 noise)
  - channel stats via DVE tensor_tensor_reduce: one pass per t computes
    (x_lo max x_hi) with fused max-accum -> pr_max; one TTR(add,add,
    scale=1/HW) -> pr_mean t0; t1 mean via ACT accum pass (engine balance)
  - m1 = max(ca1*x1, xc0) via fused scalar_tensor_tensor (xc1 never
    materialized); xc0 via ACT scaled copy
  - ssum: PE matmuls (ca_b^T x) into PSUM chunks at partitions {0,32,64}
    x 3 banks; GPS SWDGE scatters PSUM->conv rows with f32->bf16 cast
  - smax: m1 laid out [128, 3584] (28*128, pixel hw = 28p + j), DMA XBAR
    transpose (2 halves on SP/ACT queues) -> mT [128, 28, 128] SBUF bf16;
    DVE fold tree at 2x + small reduce -> smT [128, 28]; one affine
    scatter via (y h) j view = [64, 56]
  - conv 7x7 as 7 accumulating PE matmuls (banded lhsT, 1/C folded in)
  - sa broadcast via GPSIMD partition_broadcast in two halves
  - final: ob = (x*ca)*sab via 4 quarter scalar_tensor_tensor ops
  - stores: t0 on GPSIMD SWDGE, t1 on ACT HWDGE queue
"""

import sys
from contextlib import ExitStack

import numpy as np

if "/opt/trn_rl_repo" not in sys.path:
    sys.path.insert(0, "/opt/trn_rl_repo")

import concourse.bass as bass
import concourse.tile as tile
from concourse import bacc, mybir
from concourse.bass_utils import run_bass_kernel_spmd
from concourse.masks import make_identity

F32 = mybir.dt.float32
BF16 = mybir.dt.bfloat16
ALU = mybir.AluOpType
ACTF = mybir.ActivationFunctionType

N_CORES = 8
B, T, C, H, W = 8, 8, 256, 56, 56
HW = H * W            # 3136
FRAMES = B * T        # 64
FPC = FRAMES // N_CORES  # frames per core = 8
PAD = 3
HP, WP = H + 2 * PAD, W + 2 * PAD  # 62, 62
SCHK = 392            # ssum chunk width (8 chunks)
HHW = HW // 2         # 1568
MPAD = 28 * 128       # 3584: m1 padded so hw = 28*p + j transposes cleanly
NEG = -3.0e38


def _build_conv_lhsT(conv_w: np.ndarray) -> np.ndarray:
    """Banded matrices for the 7x7 conv as accumulating matmuls.

    Branch 0 (avg, y-banded, 1/C folded): B[0, dx][yi, yo] =
    w_eff[0, yi-yo, dx] -- contracts y_in for each of 7 dx columns.
    Branch 1 (max, x-banded, operates on the transposed max map):
    B[1, dy][xi, xo] = w_eff[1, dy, xi-xo] -- contracts x_in for each
    of 7 dy rows; its [x_out, y_out] result is transposed back on PE.
    """
    w_eff = conv_w[0].astype(np.float64).copy()  # [2, 7, 7]
    w_eff[0] /= C
    Bm = np.zeros((2, 7, HP, H), dtype=np.float32)
    di = np.arange(7)
    for d in range(7):
        for o in range(H):
            Bm[0, d, o + di, o] = w_eff[0, :, d]   # dx = d, band over y
            Bm[1, d, o + di, o] = w_eff[1, d, :]   # dy = d, band over x
    return Bm


def build_nc(n_frames: int = FPC):
    nc = bacc.Bacc("TRN2", target_bir_lowering=False, debug=False,
                   num_devices=N_CORES)

    x_ext = nc.dram_tensor("x", [n_frames, C, HW], BF16, kind="ExternalInput")
    w1_ext = nc.dram_tensor("w1", [C, 16], F32, kind="ExternalInput")
    w2_ext = nc.dram_tensor("w2", [16, C], F32, kind="ExternalInput")
    cb_ext = nc.dram_tensor("convb", [2, 7, HP, H], F32, kind="ExternalInput")
    out_ext = nc.dram_tensor("out", [n_frames, C, HW], BF16,
                             kind="ExternalOutput")

    with tile.TileContext(nc) as tc, ExitStack() as ctx:
        consts = ctx.enter_context(tc.tile_pool(name="consts", bufs=1))
        xin = ctx.enter_context(tc.tile_pool(name="xin", bufs=3))
        scrp = ctx.enter_context(tc.tile_pool(name="scr", bufs=1))
        xcp = ctx.enter_context(tc.tile_pool(name="xc", bufs=2))
        m1p = ctx.enter_context(tc.tile_pool(name="m1", bufs=2))
        mtp = ctx.enter_context(tc.tile_pool(name="mt", bufs=2))
        foldp = ctx.enter_context(tc.tile_pool(name="fold", bufs=1))
        smtp = ctx.enter_context(tc.tile_pool(name="smt", bufs=2))
        sabp = ctx.enter_context(tc.tile_pool(name="sab", bufs=2))
        obp = ctx.enter_context(tc.tile_pool(name="ob", bufs=2))
        sap = ctx.enter_context(tc.tile_pool(name="sa", bufs=2))
        small = ctx.enter_context(tc.tile_pool(name="small", bufs=3))
        # PSUM: pss 4 banks + misc 2 banks = 6
        pssp = ctx.enter_context(tc.tile_pool(name="pss", bufs=1, space="PSUM"))
        pmp = ctx.enter_context(tc.tile_pool(name="pm", bufs=2, space="PSUM"))

        # ---- constants / weights (loaded once) ----
        w1_sb = consts.tile([128, 2, 16], F32)       # [k, ktile, m]
        for t in range(2):
            nc.sync.dma_start(w1_sb[:, t, :], w1_ext[t * 128:(t + 1) * 128, :])
        w2_sb = consts.tile([16, C], F32)
        nc.sync.dma_start(w2_sb[:], w2_ext[:, :])
        cb_sb = consts.tile([HP, 2, 7, H], BF16)     # [in_row, branch, d, out]
        nc.gpsimd.dma_start(                          # SWDGE: casts f32->bf16
            cb_sb[:],
            cb_ext.rearrange("b d p y -> p b d y"),
        )
        ident = consts.tile([64, 64], BF16)
        make_identity(nc, ident[:])

        xbs = {}
        a_st = {}
        b_st = {}

        def load_frame(g):
            if g >= n_frames:
                return
            xb = xin.tile([128, 2, HW], BF16, tag="x")
            for t in range(2):
                nc.sync.dma_start(
                    xb[:, t, :], x_ext[g, t * 128:(t + 1) * 128, :])
            xbs[g] = xb

        def stage_a(g):
            """stats -> MLP -> ca -> xc -> m1 -> ssum -> drain -> XBAR."""
            if g >= n_frames:
                return
            load_frame(g + 2)
            xb = xbs.pop(g)

            # stats: max fold tree (DVE); means via ACT accumulate
            pr_max = small.tile([128, 2], F32, tag="prmax")
            pr_sum = small.tile([128, 2], F32, tag="prsum")
            f1 = foldp.tile([128, 2, HHW], BF16, tag="f1")
            nc.vector.tensor_tensor(
                out=f1[:], in0=xb[:, :, 0:HHW], in1=xb[:, :, HHW:HW],
                op=ALU.max)
            f2 = foldp.tile([128, 2, 784], BF16, tag="f2")
            nc.vector.tensor_tensor(
                out=f2[:], in0=f1[:, :, 0:784], in1=f1[:, :, 784:HHW],
                op=ALU.max)
            f3 = foldp.tile([128, 2, 392], BF16, tag="f3")
            nc.vector.tensor_tensor(
                out=f3[:], in0=f2[:, :, 0:392], in1=f2[:, :, 392:784],
                op=ALU.max)
            nc.vector.tensor_reduce(
                out=pr_max[:], in_=f3[:],
                axis=mybir.AxisListType.X, op=ALU.max)
            scr1 = scrp.tile([128, 2, HW], BF16, tag="scr1")
            for t in range(2):
                nc.scalar.activation(
                    scr1[:, t, :], xb[:, t, :], ACTF.Copy, scale=1.0 / HW,
                    accum_out=pr_sum[:, t:t + 1])

            # MLP on PE (stats are already mean-scaled)
            ph = pmp.tile([16, 2], F32, tag="misc")
            for si, prs in ((0, pr_max), (1, pr_sum)):
                for t in range(2):
                    nc.tensor.matmul(ph[:, si:si + 1], w1_sb[:, t, :],
                                     prs[:, t:t + 1],
                                     start=(t == 0), stop=(t == 1))
            h = small.tile([16, 2], F32, tag="h")
            nc.scalar.activation(h[:], ph[:], ACTF.Relu)
            hs = small.tile([16, 1], F32, tag="hs")
            nc.vector.tensor_tensor(out=hs[:], in0=h[:, 0:1], in1=h[:, 1:2],
                                    op=ALU.add)
            pca = pmp.tile([128, 2], F32, tag="misc")
            for t in range(2):
                nc.tensor.matmul(pca[:, t:t + 1],
                                 w2_sb[:, t * 128:(t + 1) * 128], hs[:],
                                 start=True, stop=True)
            ca = small.tile([128, 2], F32, tag="ca")
            nc.scalar.activation(ca[:], pca[:], ACTF.Sigmoid)
            ca_b = small.tile([128, 2], BF16, tag="ca_b")
            nc.scalar.activation(ca_b[:], pca[:], ACTF.Sigmoid)

            # xc0 on ACT, xc1 on DVE; m1 = max(xc0, xc1) in 64y+x layout
            xc0 = xcp.tile([128, HW], BF16, tag="xc0")
            nc.scalar.activation(xc0[:], xb[:, 0, :], ACTF.Copy,
                                 scale=ca[:, 0:1])
            xc1 = xcp.tile([128, HW], BF16, tag="xc1")
            nc.vector.tensor_scalar(
                out=xc1[:], in0=xb[:, 1, :],
                scalar1=ca[:, 1:2], scalar2=None, op0=ALU.mult)
            m1 = m1p.tile([128, MPAD], BF16, tag="m1")
            m1_v = m1[:].rearrange("p (y q) -> p y q", q=64)
            nc.gpsimd.memset(m1_v[:, :, H:64], 0.0)
            nc.vector.tensor_tensor(
                out=m1_v[:, :, 0:H],
                in0=xc0[:].rearrange("p (y x) -> p y x", x=W),
                in1=xc1[:].rearrange("p (y x) -> p y x", x=W),
                op=ALU.max)

            # ssum via PE: 8 chunks of 392 at {0,32} x 4 banks, M=32
            pss = pssp.tile([64, 4, 512], F32, tag="pss")
            for t in range(2):
                ca_col = ca_b[:, t:t + 1]
                ca_m32 = bass.AP(
                    ca_col.tensor, ca_col.offset,
                    type(ca_col.ap)([list(ca_col.ap[0]), [0, 32]]))
                for j in range(8):
                    bp, bk = 32 * (j // 4), j % 4
                    nc.tensor.matmul(
                        pss[bp:bp + 32, bk, 0:SCHK],
                        ca_m32,
                        xb[:, t, j * SCHK:(j + 1) * SCHK],
                        start=(t == 0), stop=(t == 1),
                        skip_group_check=True)
            ssb = sap.tile([64, 4, SCHK], BF16, tag="ssb")
            nc.scalar.activation(ssb[:], pss[:, :, 0:SCHK], ACTF.Copy)

            # smax transpose: HW XBAR semantic out[a, b, c] = in[c,
            # 128b + a] (verified on device). With the 64y+x layout:
            # mT[64h + x, q, c] = pixel(y = 2q + h, x) of channel c.
            # Concurrent XBAR transposes corrupt each other (shared ucode
            # state) -- all transposes stay on the sync queue.
            mT = mtp.tile([128, 28, 128], BF16, tag="mT")
            nc.sync.dma_start_transpose(mT[:, :, :], m1[:])

            a_st[g] = (xc0, xc1, mT, ssb)

        def stage_b(g):
            """folds -> smT -> conv assembly -> conv -> sa -> broadcast."""
            xc0, xc1, mT, ssb = a_st.pop(g)
            sf1 = foldp.tile([128, 28, 64], BF16, tag="sf1")
            nc.vector.tensor_tensor(
                out=sf1[:], in0=mT[:, :, 0:64], in1=mT[:, :, 64:128],
                op=ALU.max)
            sf2 = foldp.tile([128, 28, 32], BF16, tag="sf2")
            nc.vector.tensor_tensor(
                out=sf2[:], in0=sf1[:, :, 0:32], in1=sf1[:, :, 32:64],
                op=ALU.max)
            sf3 = foldp.tile([128, 28, 16], BF16, tag="sf3")
            nc.vector.tensor_tensor(
                out=sf3[:], in0=sf2[:, :, 0:16], in1=sf2[:, :, 16:32],
                op=ALU.max)
            smT = smtp.tile([128, 28], BF16, tag="smT")
            nc.vector.tensor_reduce(
                out=smT[:], in_=sf3[:],
                axis=mybir.AxisListType.X, op=ALU.max)

            # conv input assembly
            # avg map, y-banded: chunk j=(p/32)*4+bank holds rows 7j..7j+6
            sp_avg = sap.tile([HP, WP], BF16, tag="sp_avg")
            nc.gpsimd.memset(sp_avg[:], 0.0)
            nc.gpsimd.dma_start(sp_avg[PAD:PAD + H, PAD:PAD + W],
                                ssb[0:33:32, :, :])
            # max map, TRANSPOSED (x-banded): smT[64h + x, j] =
            # maxpixel(y = 2j + h, x) -> stride-2 column interleave
            sp_maxT = sap.tile([HP, WP], BF16, tag="sp_maxT")
            nc.gpsimd.memset(sp_maxT[:], 0.0)
            nc.gpsimd.dma_start(sp_maxT[PAD:PAD + H, PAD:PAD + H:2],
                                smT[0:H, :])
            nc.gpsimd.dma_start(sp_maxT[PAD:PAD + H, PAD + 1:PAD + H + 1:2],
                                smT[64:64 + H, :])

            # conv: max branch x-banded into its own PSUM bank (transposed
            # result), avg branch y-banded; PE transpose merges them
            pcvT = pmp.tile([H, W], F32, tag="misc")
            for dy in range(7):
                nc.tensor.matmul(pcvT[:], cb_sb[:, 1, dy, :],
                                 sp_maxT[:, dy:dy + W],
                                 start=(dy == 0), stop=(dy == 6))
            cvT_b = small.tile([H, W], BF16, tag="cvT_b")
            nc.scalar.activation(cvT_b[:], pcvT[:], ACTF.Copy)
            pcv = pmp.tile([H, W], F32, tag="misc")
            for dx in range(7):
                nc.tensor.matmul(pcv[:], cb_sb[:, 0, dx, :],
                                 sp_avg[:, dx:dx + W],
                                 start=(dx == 0), stop=False)
            nc.tensor.matmul(pcv[:], cvT_b[:], ident[0:H, 0:W],
                             start=False, stop=True)
            sa_yx = small.tile([H, W], BF16, tag="sa_yx")
            nc.scalar.activation(sa_yx[:], pcv[:], ACTF.Sigmoid)

            # sa broadcast on GPSIMD
            sa_row = sap.tile([1, HW], BF16, tag="sa_row")
            nc.scalar.dma_start(sa_row[:], sa_yx[:])
            sab = sabp.tile([128, HW], BF16, tag="sab")
            nc.gpsimd.partition_broadcast(sab[:], sa_row[0:1, :],
                                          channels=128)
            b_st[g] = (xc0, xc1, sab)

        def emit_ob(g):
            if g < 0 or g not in b_st:
                return
            xc0, xc1, sab = b_st.pop(g)
            ob = obp.tile([128, 2, HW], BF16, tag="ob")
            for t, xc in ((0, xc0), (1, xc1)):
                nc.vector.tensor_tensor(
                    out=ob[:, t, :], in0=xc[:], in1=sab[:], op=ALU.mult)
                eng = nc.gpsimd if t == 0 else nc.sync
                eng.dma_start(
                    out_ext[g, t * 128:(t + 1) * 128, :], ob[:, t, :])

        load_frame(0)
        load_frame(1)
        stage_a(0)
        for f in range(n_frames):
            stage_a(f + 1)
            emit_ob(f - 1)
            stage_b(f)
        emit_ob(n_frames - 1)

    nc.finalize()
    return nc


_NC_CACHE = {}


def _get_nc(n_frames: int):
    if n_frames not in _NC_CACHE:
        _NC_CACHE[n_frames] = build_nc(n_frames)
    return _NC_CACHE[n_frames]


def _make_in_maps(f, w1, w2, conv_w):
    import ml_dtypes
    w1 = np.ascontiguousarray(np.asarray(w1, dtype=np.float32))
    w2 = np.ascontiguousarray(np.asarray(w2, dtype=np.float32))
    conv_w = np.asarray(conv_w, dtype=np.float32)
    convb = _build_conv_lhsT(conv_w)
    frames = np.asarray(f, dtype=np.float32).reshape(FRAMES, C, HW)
    frames = frames.astype(ml_dtypes.bfloat16)
    in_maps = []
    for i in range(N_CORES):
        in_maps.append({
            "x": np.ascontiguousarray(frames[i * FPC:(i + 1) * FPC]),
            "w1": w1,
            "w2": w2,
            "convb": convb,
        })
    return in_maps


def kernel(f: np.ndarray, w1: np.ndarray, w2: np.ndarray,
           conv_w: np.ndarray) -> np.ndarray:
    in_maps = _make_in_maps(f, w1, w2, conv_w)
    nc = _get_nc(FPC)
    res = run_bass_kernel_spmd(nc, in_maps, core_ids=list(range(N_CORES)))
    out = np.concatenate(
        [np.asarray(res.results[i]["out"]).astype(np.float32)
         for i in range(N_CORES)], axis=0)
    return out.reshape(B, T, C, H, W)


if __name__ == "__main__":
    rng = np.random.default_rng(0)
    f = rng.standard_normal((B, T, C, H, W), dtype=np.float32)
    w1 = rng.standard_normal((C, 16), dtype=np.float32) / 16.0
    w2 = rng.standard_normal((16, C), dtype=np.float32) / 4.0
    conv_w = rng.standard_normal((1, 2, 7, 7), dtype=np.float32) * 0.1
    out = kernel(f, w1, w2, conv_w)
    print("kernel ran, out shape", out.shape, out.dtype)
